# revision 21
# baseline (speedup 1.0000x reference)
"""DFlash draft-model kernel for 8x Trainium2 NeuronCores.

Sharding: head-parallel attention (core c owns head c) + vocab-parallel LM head
(core c owns vocab shard c), joined by an AllGather of the normalized per-head
context (fp8). Block-sparse attention: kv tiles above each q-group's max
anchor are skipped; within scheduled tiles, dead query columns (sorted
anchors => dead prefix) are trimmed from the score matmul / exp / PV.

fp8 (e4m3) DoubleRow matmuls carry projections, PV, Wo and the LM head;
scales are folded into activation scales and the softmax reciprocal
(ones-column = 2.0 => denom = 2*sum(p), recip * ctx*32 = ctx*16).

Per-core device outputs: row sum(exp) (f32) + row max(exp) (bf16-accurate) of
its logit shard, and the target-column logits; host combines into
(loss, accuracy).
"""
import sys
sys.path.insert(0, '/opt/trn_rl_repo')
import numpy as np
import ml_dtypes

import concourse.mybir as mybir
import concourse.tile as tile
from concourse import bacc
from concourse.bass_utils import run_bass_kernel_spmd
from concourse.bass_interp import get_hw_module

F32 = mybir.dt.float32
BF16 = mybir.dt.bfloat16
FP16 = mybir.dt.float16
F8 = mybir.dt.float8e4
BFNP = ml_dtypes.bfloat16
F8NP = ml_dtypes.float8_e4m3
DR = mybir.MatmulPerfMode.DoubleRow

B, S, N, BS, D, H, V = 1, 2048, 128, 16, 512, 8, 32000
MASK_TOKEN_ID = 3
NC = 8
DH = D // H            # 64
Q = N * BS             # 2048
VS = V // NC           # 4000 vocab per core
QG = 4                 # q groups of 512
ZC = 500               # logits psum chunk (bank-aligned at 512 offsets)

# fp8 scale plan (see module docstring)
SX = 16.0   # x (hidden/noise-emb) scale
SW = 32.0   # Wq/Wk/Wv scale
SK = 8.0    # k/q storage scale -> scores psum = s*64
SV = 32.0   # v scale in vaug
SG = 16.0   # gathered ctx scale (via ones-col 2.0)
SWO = 32.0  # Wo scale -> Wo psum = out*512
SO = 8.0    # outT storage scale
SL = 32.0   # W_lm scale -> lm psum = z*256

_cache = {}
_last_in_maps = None


def _build_schedule(anc):
    """Per q-group: ctx kv tile pairs [(t, masked, bmask_idx)], pair dead-col a0."""
    sched = []
    nmask = 0
    for g in range(QG):
        blk = anc[32 * g:32 * g + 32]          # anchors of this group's blocks
        amin, amax = int(blk.min()), int(blk.max())
        ctx = []
        for t in range((amax + 127) // 128):
            masked = (128 * t + 128) > amin
            # dead prefix: q columns whose anchor <= 128*t (block granular)
            a = int((blk <= 128 * t).sum()) * BS
            mi = -1
            if masked:
                mi = nmask
                nmask += 1
            ctx.append((t, 1 if masked else 0, a, mi))
        pairs = [ctx[i:i + 2] for i in range(0, len(ctx), 2)]
        sched.append(pairs)
    return sched, nmask


def _build_program(sched, nmask, reps=1, collective=True):
    nc = bacc.Bacc("TRN2", target_bir_lowering=False, debug=False, num_devices=NC)

    din = {}
    for name, shape, dt in [
        ("i_bmask", [128, nmask * 512], F8),  # boundary masks (anchor > kv)
        ("i_dmask", [128, 512], F8),       # draft block-diag pattern x4
        ("i_wq", [128, 256], F8),          # [p, fp*128 + j*64 + d]
        ("i_wk", [128, 256], F8),
        ("i_wv", [128, 256], F8),
        ("i_xt0", [128, 2 * (S + Q)], F8),  # X^T*SX rows 0..255   [p, j*4096+x]
        ("i_xt1", [128, 2 * (S + Q)], F8),  # rows 256..511
        ("i_wo0", [128, 2 * D], F8),       # [p, j*512 + o]
        ("i_wo1", [128, 2 * D], F8),
        ("i_wt", [128, 4 * Q], BF16),      # Wlm[:,targets]/SO  [p, f*2048+q]
        ("i_wlm0", [128, 2 * VS], F8),     # [p, j*4000 + v]
        ("i_wlm1", [128, 2 * VS], F8),
    ]:
        din[name] = nc.dram_tensor(name, shape, dt, kind="ExternalInput").ap()
    o_se = nc.dram_tensor("o_se", [128, 16], F32, kind="ExternalOutput").ap()
    o_mx = nc.dram_tensor("o_mx", [128, 16], F32, kind="ExternalOutput").ap()
    o_tl = nc.dram_tensor("o_tl", [1, Q], F32, kind="ExternalOutput").ap()

    with tile.TileContext(nc) as tc:
        for _rep in range(reps):
            _emit(nc, tc, din, o_se, o_mx, o_tl, sched, nmask, collective, _rep)

    nc.compile()
    nc.m = get_hw_module(nc.m)
    return nc


def _emit(nc, tc, din, o_se, o_mx, o_tl, sched, nmask, collective, rep):
    with tc.tile_pool(name=f"persist{rep}", bufs=1) as pp, \
         tc.tile_pool(name=f"dram{rep}", bufs=1, space="DRAM") as dp:
        # ---- loads: projection inputs first, masks next, lm-head weights last
        wq_sb = pp.tile([128, 256], F8, name="wq_sb")
        nc.sync.dma_start(wq_sb[:], din["i_wq"][:])
        wk_sb = pp.tile([128, 256], F8, name="wk_sb")
        nc.sync.dma_start(wk_sb[:], din["i_wk"][:])
        wv_sb = pp.tile([128, 256], F8, name="wv_sb")
        nc.sync.dma_start(wv_sb[:], din["i_wv"][:])
        xt = []
        for f in range(2):
            t = pp.tile([128, 2 * (S + Q)], F8, name=f"xt{f}")
            nc.sync.dma_start(t[:], din[f"i_xt{f}"][:])
            xt.append(t)
        bmask = pp.tile([128, max(1, nmask) * 512], F8, name="bmask")
        nc.sync.dma_start(bmask[:], din["i_bmask"][:])
        dmask = pp.tile([128, 512], F8, name="dmask")
        nc.sync.dma_start(dmask[:], din["i_dmask"][:])
        wo = []
        for f in range(2):
            t = pp.tile([128, 2 * D], F8, name=f"wo{f}")
            nc.sync.dma_start(t[:], din[f"i_wo{f}"][:])
            wo.append(t)
        wt_sb = pp.tile([128, 4 * Q], BF16, name="wt_sb")
        nc.sync.dma_start(wt_sb[:], din["i_wt"][:])
        wlm = []
        for f in range(2):
            t = pp.tile([128, 2 * VS], F8, name=f"wlm{f}")
            nc.sync.dma_start(t[:], din[f"i_wlm{f}"][:])
            wlm.append(t)

        xtv = [t.rearrange("p (j x) -> p j x", j=2) for t in xt]
        wqv = wq_sb.rearrange("p (f j d) -> p f j d", f=2, j=2)
        wkv = wk_sb.rearrange("p (f j d) -> p f j d", f=2, j=2)
        wvv = wv_sb.rearrange("p (f j d) -> p f j d", f=2, j=2)
        wov = [t.rearrange("p (j o) -> p j o", j=2) for t in wo]
        wlmv = [t.rearrange("p (j v) -> p j v", j=2) for t in wlm]
        wtv = wt_sb.rearrange("p (f q) -> p f q", f=4)

        ones64 = pp.tile([1, DH], BF16, name="ones64")
        nc.vector.memset(ones64[:], 1.0)
        onescol = pp.tile([128, 1], BF16, name="onescol")
        nc.vector.memset(onescol[:], 1.0)

        kT = pp.tile([DH, S + Q], BF16, name="kT")
        qT = pp.tile([DH, Q], BF16, name="qT")
        vaug = pp.tile([128, 32 * 68], F8, name="vaug")
        vav = vaug.rearrange("p (t c) -> p t c", c=68)
        nc.vector.memset(vav[:, :, DH:DH + 1], 2.0)   # denom ones-col (=2)
        nc.vector.memset(vav[:, :, DH + 1:68], 0.0)   # fp8-align padding
        gin = pp.tile([DH, Q], F8, name="gin")
        recip = pp.tile([1, Q], BF16, name="recip")
        bcs = pp.tile([DH, 512], BF16, name="bcs")
        ctxf = [pp.tile([128, 2 * Q], F8, name=f"ctxf{f}") for f in range(2)]
        ctxfv = [t.rearrange("p (j q) -> p j q", j=2) for t in ctxf]
        outT = [pp.tile([128, 2 * Q], F8, name=f"outT{f}") for f in range(2)]
        outTv = [t.rearrange("p (j q) -> p j q", j=2) for t in outT]
        se_sb = pp.tile([128, 16], F32, name="se_sb")
        mx_sb = pp.tile([128, 16], F32, name="mx_sb")
        tl_sb = pp.tile([1, Q], F32, name="tl_sb")
        gb_in = [dp.tile([DH, Q // 2], F8, name=f"gb_in{h}") for h in range(2)]
        gb_out = [dp.tile([NC * DH, Q // 2], F8, name=f"gb_out{h}",
                          addr_space="Shared" if collective else "Local")
                  for h in range(2)]

        # ---- projections (DoubleRow fp8)
        with tc.tile_pool(name=f"projps{rep}", bufs=2, space="PSUM") as projps:
            for n in range((S + Q) // 512):
                ps = projps.tile([DH, 512], F32, name="kps", tag="proj")
                for f in range(2):
                    nc.tensor.matmul(ps[:], wkv[:, f], xtv[f][:, :, 512 * n:512 * (n + 1)],
                                     start=(f == 0), stop=(f == 1), perf_mode=DR)
                nc.vector.tensor_scalar(kT[:, 512 * n:512 * (n + 1)], ps[:],
                                        SK / (SX * SW), None, mybir.AluOpType.mult)
            for n in range(Q // 512):
                ps = projps.tile([DH, 512], F32, name="qps", tag="proj")
                for f in range(2):
                    nc.tensor.matmul(ps[:], wqv[:, f],
                                     xtv[f][:, :, S + 512 * n:S + 512 * (n + 1)],
                                     start=(f == 0), stop=(f == 1), perf_mode=DR)
                nc.vector.tensor_scalar(qT[:, 512 * n:512 * (n + 1)], ps[:],
                                        SK / (SX * SW), None, mybir.AluOpType.mult)
            for T in range(32):
                ps = projps.tile([128, DH], F32, name="vps", tag="proj")
                for f in range(2):
                    nc.tensor.matmul(ps[:], xtv[f][:, :, 128 * T:128 * (T + 1)],
                                     wvv[:, f], start=(f == 0), stop=(f == 1),
                                     perf_mode=DR)
                nc.scalar.mul(vav[:, T, 0:DH], ps[:], SV / (SX * SW))

        # ---- attention: two-half pipeline with per-half AllGather
        with tc.tile_pool(name=f"scoreps{rep}", bufs=2, space="PSUM") as scoreps, \
             tc.tile_pool(name=f"ctxps{rep}", bufs=3, space="PSUM") as ctxps, \
             tc.tile_pool(name=f"bcps{rep}", bufs=1, space="PSUM") as bcps, \
             tc.tile_pool(name=f"abuf{rep}", bufs=4) as abuf:
            for half in range(2):
                for g in (2 * half, 2 * half + 1):
                    pairs = sched[g]
                    qs = 512 * g
                    cps = ctxps.tile([68, 512], F32, name="cps")
                    # draft tiles first (start=True per 128-col region)
                    dps = scoreps.tile([128, 512], F32, name="dsps", tag="sc")
                    for u in range(4):
                        t = 16 + 4 * g + u
                        nc.tensor.matmul(dps[:, 128 * u:128 * (u + 1)],
                                         kT[:, 128 * t:128 * (t + 1)],
                                         qT[:, qs + 128 * u:qs + 128 * (u + 1)],
                                         start=True, stop=True)
                    dp_sb = abuf.tile([128, 1024], F8, name="p_sb", tag="p")
                    nc.scalar.activation(dp_sb[:, 0:512], dps[:],
                                         mybir.ActivationFunctionType.Exp,
                                         scale=0.125 / (SK * SK))
                    nc.gpsimd.tensor_tensor(dp_sb[:, 0:512], dp_sb[:, 0:512],
                                            dmask[:], mybir.AluOpType.mult)
                    for u in range(4):
                        t = 16 + 4 * g + u
                        nc.tensor.matmul(cps[:, 128 * u:128 * (u + 1)],
                                         vav[:, t], dp_sb[:, 128 * u:128 * (u + 1)],
                                         start=True, stop=True,
                                         skip_group_check=True)
                    # ctx tiles in pairs, trimmed to live cols
                    for pi, pair in enumerate(pairs):
                        a0 = pair[0][2]
                        sps = scoreps.tile([128, 1024], F32, name="sps", tag="sc")
                        for m, (t, mtype, a, mi) in enumerate(pair):
                            nc.tensor.matmul(sps[:, 512 * m + a:512 * m + 512],
                                             kT[:, 128 * t:128 * (t + 1)],
                                             qT[:, qs + a:qs + 512],
                                             start=True, stop=True)
                        p_sb = abuf.tile([128, 1024], F8, name="p_sb", tag="p")
                        spv = sps.rearrange("p (m w) -> p m w", m=2)
                        ppv = p_sb.rearrange("p (m w) -> p m w", m=2)
                        nc.scalar.activation(ppv[:, 0:len(pair), a0:512],
                                             spv[:, 0:len(pair), a0:512],
                                             mybir.ActivationFunctionType.Exp,
                                             scale=0.125 / (SK * SK))
                        for m, (t, mtype, a, mi) in enumerate(pair):
                            if mtype:
                                pv = p_sb[:, 512 * m + a0:512 * m + 512]
                                nc.gpsimd.tensor_tensor(
                                    pv, pv, bmask[:, 512 * mi + a0:512 * mi + 512],
                                    mybir.AluOpType.mult)
                        for m, (t, mtype, a, mi) in enumerate(pair):
                            nc.tensor.matmul(cps[:, a0:512],
                                             vav[:, t],
                                             p_sb[:, 512 * m + a0:512 * m + 512],
                                             start=False,
                                             stop=(pi == len(pairs) - 1
                                                   and m == len(pair) - 1),
                                             skip_group_check=True)
                    # normalize: recip of (2*sum p) => ctx*SG via *32 values
                    with nc.allow_low_precision(reason="bf16 recip of denom"):
                        nc.vector.reciprocal(recip[:, qs:qs + 512],
                                             cps[DH:DH + 1, :])
                    bps = bcps.tile([DH, 512], F32, name="bps")
                    nc.tensor.matmul(bps[:], ones64[:], recip[:, qs:qs + 512],
                                     start=True, stop=True)
                    nc.vector.tensor_copy(bcs[:], bps[:])
                    nc.vector.tensor_tensor(gin[:, qs:qs + 512], cps[0:DH, :],
                                            bcs[:], mybir.AluOpType.mult)
                # AllGather for this half
                hs_ = slice(1024 * half, 1024 * (half + 1))
                nc.sync.dma_start(gb_in[half][:], gin[:, hs_])
                if collective:
                    nc.gpsimd.collective_compute(
                        "AllGather", mybir.AluOpType.bypass,
                        replica_groups=[list(range(NC))],
                        ins=[gb_in[half].opt()], outs=[gb_out[half].opt()])
                else:  # timing-model variant: fake the gather with local DMAs
                    for _c in range(NC):
                        nc.sync.dma_start(gb_out[half][DH * _c:DH * (_c + 1), :],
                                          gb_in[half][:])
                for f in range(2):
                    for j in range(2):
                        nc.sync.dma_start(
                            ctxfv[f][:, j, 1024 * half:1024 * (half + 1)],
                            gb_out[half][256 * f + 128 * j:256 * f + 128 * (j + 1), :])

        # ---- Wo (DoubleRow fp8) + target logits, all 4 groups
        with tc.tile_pool(name=f"wops{rep}", bufs=2, space="PSUM") as wops, \
             tc.tile_pool(name=f"tlps{rep}", bufs=2, space="PSUM") as tlps, \
             tc.tile_pool(name=f"stbuf{rep}", bufs=2) as stbuf:
            for g in range(QG):
                for fo in range(4):
                    ps = wops.tile([128, 512], F32, name="wps")
                    for f in range(2):
                        nc.tensor.matmul(
                            ps[:], wov[f][:, :, 128 * fo:128 * (fo + 1)],
                            ctxfv[f][:, :, 512 * g:512 * (g + 1)],
                            start=(f == 0), stop=(f == 1), perf_mode=DR)
                    nc.vector.tensor_scalar(
                        outTv[fo // 2][:, fo % 2, 512 * g:512 * (g + 1)],
                        ps[:], SO / (SG * SWO), None, mybir.AluOpType.mult)
            for g in range(QG):
                ps = tlps.tile([1, 512], F32, name="tlp")
                for fo in range(4):
                    mmc = stbuf.tile([128, 512], BF16, name="mmc", tag="mmc")
                    nc.gpsimd.tensor_tensor(
                        mmc[:], outTv[fo // 2][:, fo % 2, 512 * g:512 * (g + 1)],
                        wtv[:, fo, 512 * g:512 * (g + 1)], mybir.AluOpType.mult)
                    nc.tensor.matmul(ps[:], onescol[:], mmc[:],
                                     start=(fo == 0), stop=(fo == 3))
                nc.vector.tensor_copy(tl_sb[:, 512 * g:512 * (g + 1)], ps[:])

        # ---- LM head: 16 q-chunks x 4000 vocab, fp8 DoubleRow, exp+accum,
        #      bf16 tree-max (DVE 2x; Pool helps on odd chunks)
        with tc.tile_pool(name=f"zps{rep}", bufs=2, space="PSUM") as zps, \
             tc.tile_pool(name=f"zbuf{rep}", bufs=4) as zbuf, \
             tc.tile_pool(name=f"mxbuf{rep}", bufs=4) as mxbuf, \
             tc.tile_pool(name=f"stbuf2{rep}", bufs=2) as stbuf2:
            for i in range(16):
                se2 = stbuf2.tile([128, 2], F32, name="se2", tag="se2")
                zes = []
                for c2 in range(2):
                    ps = zps.tile([128, 2048], F32, name="zp")
                    for f in range(2):
                        for h in range(4):
                            nc.tensor.matmul(
                                ps[:, 512 * h:512 * h + ZC],
                                outTv[f][:, :, 128 * i:128 * (i + 1)],
                                wlmv[f][:, :, 2000 * c2 + ZC * h:2000 * c2 + ZC * (h + 1)],
                                start=(f == 0), stop=(f == 1),
                                perf_mode=DR, skip_group_check=True)
                    ze = zbuf.tile([128, 2048], BF16, name="ze")
                    psv = ps.rearrange("p (c w) -> p c w", w=512)[:, :, 0:ZC]
                    zev = ze.rearrange("p (c w) -> p c w", w=512)[:, :, 0:ZC]
                    nc.scalar.activation(zev, psv,
                                         mybir.ActivationFunctionType.Exp,
                                         scale=1.0 / (SO * SL),
                                         accum_out=se2[:, c2:c2 + 1])
                    zes.append(ze)
                nc.vector.tensor_tensor(se_sb[:, i:i + 1], se2[:, 0:1], se2[:, 1:2],
                                        mybir.AluOpType.add)
                # tree max over 2x[128, 4x500(str512)] bf16
                zm = [mxbuf.tile([128, 1024], BF16, name="zm", tag="zm")
                      for _ in range(2)]
                for c2 in range(2):
                    zv = zes[c2].rearrange("p (c w) -> p c w", w=512)[:, :, 0:ZC]
                    zmv = zm[c2].rearrange("p (c w) -> p c w", w=512)[:, :, 0:ZC]
                    nc.vector.tensor_tensor(zmv, zv[:, 0:2], zv[:, 2:4],
                                            mybir.AluOpType.max)
                z3 = zm[0].rearrange("p (c w) -> p c w", w=512)[:, :, 0:ZC]
                z4 = zm[1].rearrange("p (c w) -> p c w", w=512)[:, :, 0:ZC]
                nc.vector.tensor_tensor(z3, z3, z4, mybir.AluOpType.max)
                nc.vector.tensor_tensor(zm[0][:, 0:ZC], zm[0][:, 0:ZC],
                                        zm[0][:, 512:512 + ZC],
                                        mybir.AluOpType.max)
                nc.vector.tensor_reduce(mx_sb[:, i:i + 1], zm[0][:, 0:ZC],
                                        mybir.AxisListType.X, mybir.AluOpType.max)
        nc.sync.dma_start(o_tl[:], tl_sb[:])
        nc.sync.dma_start(o_se[:], se_sb[:])
        nc.sync.dma_start(o_mx[:], mx_sb[:])


def _pack2(a, scale):
    """[512, X] f32 -> 2 fp8 arrays [128, 2*X]: arr_fp[p, j*X+x] = a[256fp+128j+p, x]."""
    x = a.shape[1]
    r = (a * scale).reshape(2, 2, 128, x).astype(F8NP)
    return [np.ascontiguousarray(r[fp].transpose(1, 0, 2).reshape(128, 2 * x))
            for fp in range(2)]


def kernel(**inputs):
    ids = np.asarray(inputs["input_ids"])[0].astype(np.int64)        # [S]
    hs = np.asarray(inputs["hidden_states"])[0].astype(np.float32)   # [S, D]
    lmask = np.asarray(inputs["loss_mask"])[0].astype(np.float32)    # [S]
    anc = np.asarray(inputs["anchor_positions"])[0].astype(np.int64)  # [N]
    keep = np.asarray(inputs["block_keep_mask"])[0].astype(bool)     # [N]
    emb = np.asarray(inputs["embed_table"]).astype(np.float32)       # [V, D]
    Wq = np.asarray(inputs["Wq"]).astype(np.float32)
    Wk = np.asarray(inputs["Wk"]).astype(np.float32)
    Wv = np.asarray(inputs["Wv"]).astype(np.float32)
    Wo = np.asarray(inputs["Wo"]).astype(np.float32)
    Wlm = np.asarray(inputs["W_lm"]).astype(np.float32)

    # ---- host layout prep ----
    safe_anchor = np.clip(anc, 0, S - 1)
    start_tokens = np.where(keep, ids[safe_anchor], MASK_TOKEN_ID)
    ne = np.tile(emb[MASK_TOKEN_ID], (Q, 1))
    ne[0::BS] = emb[start_tokens]                   # [Q, D]
    xt_full = np.concatenate([hs, ne], 0).T         # [D, S+Q]

    offs = np.arange(BS)
    label_idx = anc[:, None] + offs[None, :]
    valid = (label_idx < S)
    safe_idx = np.clip(label_idx, 0, S - 1)
    targets = ids[safe_idx].reshape(-1)             # [Q]
    w = (keep[:, None] * valid * (offs > 0)[None, :]
         * lmask[safe_idx]).astype(np.float32).reshape(-1)

    xt8 = _pack2(xt_full, SX)
    p_idx = np.arange(128)[:, None]
    j_idx = np.arange(512)[None, :]
    dmask = ((p_idx // BS) == (j_idx % 128) // BS).astype(np.float32).astype(F8NP)
    sched, nmask = _build_schedule(anc)
    anchor_q = np.repeat(anc, BS)                    # [Q]
    bmask = np.zeros((128, max(1, nmask) * 512), np.float32)
    for g in range(QG):
        for pair in sched[g]:
            for (t, mtype, a, mi) in pair:
                if mtype:
                    kv = 128 * t + np.arange(128)[:, None]
                    av = anchor_q[None, 512 * g:512 * (g + 1)]
                    bmask[:, 512 * mi:512 * (mi + 1)] = (av > kv)
    bmask = bmask.astype(F8NP)
    wt = np.ascontiguousarray(
        (Wlm[:, targets] / SO).reshape(4, 128, Q).transpose(1, 0, 2)
        .reshape(128, 4 * Q)).astype(BFNP)
    wo8 = _pack2(Wo, SWO)
    wlm8 = {}
    for c in range(NC):
        wlm8[c] = _pack2(Wlm[:, VS * c:VS * (c + 1)], SL)

    key = (anc.tobytes(), 3)
    if key not in _cache:
        _cache[key] = _build_program(sched, nmask)
    nc = _cache[key]

    in_maps = []
    for c in range(NC):
        wq8 = _pack2(Wq[:, DH * c:DH * (c + 1)], SW)
        wk8 = _pack2(Wk[:, DH * c:DH * (c + 1)], SW)
        wv8 = _pack2(Wv[:, DH * c:DH * (c + 1)], SW)
        in_maps.append({
            "i_bmask": bmask, "i_dmask": dmask,
            "i_wq": np.concatenate(wq8, 1), "i_wk": np.concatenate(wk8, 1),
            "i_wv": np.concatenate(wv8, 1),
            "i_xt0": xt8[0], "i_xt1": xt8[1],
            "i_wo0": wo8[0], "i_wo1": wo8[1],
            "i_wt": wt,
            "i_wlm0": wlm8[c][0], "i_wlm1": wlm8[c][1],
        })

    global _last_in_maps
    _last_in_maps = in_maps
    res = run_bass_kernel_spmd(nc, in_maps, core_ids=list(range(NC)))

    # ---- host combine ----
    se = np.zeros((128, 16), np.float64)
    mx = np.zeros((128, 16), np.float32)
    for c in range(NC):
        se += res.results[c]["o_se"].astype(np.float64)
        mx = np.maximum(mx, res.results[c]["o_mx"])
    se_q = se.T.reshape(-1)           # q = 128*i + p
    mx_q = mx.T.reshape(-1)           # max of exp(z)
    tl_q = res.results[0]["o_tl"][0]

    lse = np.log(se_q)
    loss_per = np.where(w > 0, lse - tl_q, 0.0)
    loss = (loss_per * w).sum() / (w.sum() + 1e-6)
    correct = (tl_q >= np.log(np.maximum(mx_q, 1e-30)) - 3e-4) & (w > 0.5)
    acc = correct.sum() / (w.sum() + 1e-6)
    return np.float32(loss), np.float32(acc)


# revision 22
# speedup vs baseline: 1.1140x; 1.1140x over previous
"""DFlash draft-model kernel for 8x Trainium2 NeuronCores.

Sharding: head-parallel attention (core c owns head c) + vocab-parallel LM head
(core c owns vocab shard c), joined by an AllGather of the normalized per-head
context (fp8). Block-sparse attention: kv tiles above each q-group's max
anchor are skipped; within scheduled tiles, dead query columns (sorted
anchors => dead prefix) are trimmed from the score matmul / exp / PV.

fp8 (e4m3) DoubleRow matmuls carry projections, PV, Wo and the LM head;
scales are folded into activation scales and the softmax reciprocal
(ones-column = 2.0 => denom = 2*sum(p), recip * ctx*32 = ctx*16).

Per-core device outputs: row sum(exp) (f32) + row max(exp) (bf16-accurate) of
its logit shard, and the target-column logits; host combines into
(loss, accuracy).
"""
import sys
sys.path.insert(0, '/opt/trn_rl_repo')
import numpy as np
import ml_dtypes

import concourse.mybir as mybir
import concourse.tile as tile
from concourse import bacc
from concourse.bass_utils import run_bass_kernel_spmd
from concourse.bass_interp import get_hw_module

F32 = mybir.dt.float32
BF16 = mybir.dt.bfloat16
FP16 = mybir.dt.float16
F8 = mybir.dt.float8e4
BFNP = ml_dtypes.bfloat16
F8NP = ml_dtypes.float8_e4m3
DR = mybir.MatmulPerfMode.DoubleRow

B, S, N, BS, D, H, V = 1, 2048, 128, 16, 512, 8, 32000
MASK_TOKEN_ID = 3
NC = 8
DH = D // H            # 64
Q = N * BS             # 2048
VS = V // NC           # 4000 vocab per core
QG = 4                 # q groups of 512
ZC = 500               # logits psum chunk (bank-aligned at 512 offsets)

# fp8 scale plan (see module docstring)
SX = 16.0   # x (hidden/noise-emb) scale
SW = 32.0   # Wq/Wk/Wv scale
SK = 8.0    # k/q storage scale -> scores psum = s*64
SV = 32.0   # v scale in vaug
SG = 16.0   # gathered ctx scale (via ones-col 2.0)
SWO = 32.0  # Wo scale -> Wo psum = out*512
SO = 8.0    # outT storage scale
SL = 32.0   # W_lm scale -> lm psum = z*256

_cache = {}
_last_in_maps = None


def _build_schedule(anc):
    """Per q-group: ctx kv tile pairs [(t, masked, bmask_idx)], pair dead-col a0."""
    sched = []
    nmask = 0
    for g in range(QG):
        blk = anc[32 * g:32 * g + 32]          # anchors of this group's blocks
        amin, amax = int(blk.min()), int(blk.max())
        ctx = []
        for t in range((amax + 127) // 128):
            masked = (128 * t + 128) > amin
            # dead prefix: q columns whose anchor <= 128*t (block granular)
            a = int((blk <= 128 * t).sum()) * BS
            mi = -1
            if masked:
                mi = nmask
                nmask += 1
            ctx.append((t, 1 if masked else 0, a, mi))
        pairs = [ctx[i:i + 2] for i in range(0, len(ctx), 2)]
        sched.append(pairs)
    return sched, nmask


def _build_program(sched, nmask, reps=1, collective=True):
    nc = bacc.Bacc("TRN2", target_bir_lowering=False, debug=False, num_devices=NC)

    din = {}
    for name, shape, dt in [
        ("i_bmask", [128, nmask * 512], F8),  # boundary masks (anchor > kv)
        ("i_dmask", [128, 512], F8),       # draft block-diag pattern x4
        ("i_wq", [128, 256], F8),          # [p, fp*128 + j*64 + d]
        ("i_wk", [128, 256], F8),
        ("i_wv", [128, 256], F8),
        ("i_xt0", [128, 2 * (S + Q)], F8),  # X^T*SX rows 0..255   [p, j*4096+x]
        ("i_xt1", [128, 2 * (S + Q)], F8),  # rows 256..511
        ("i_wo0", [128, 2 * D], F8),       # [p, j*512 + o]
        ("i_wo1", [128, 2 * D], F8),
        ("i_wt", [128, 4 * Q], BF16),      # Wlm[:,targets]/SO  [p, f*2048+q]
        ("i_wlm0", [128, 2 * VS], F8),     # [p, j*4000 + v]
        ("i_wlm1", [128, 2 * VS], F8),
    ]:
        din[name] = nc.dram_tensor(name, shape, dt, kind="ExternalInput").ap()
    o_se = nc.dram_tensor("o_se", [128, 16], F32, kind="ExternalOutput").ap()
    o_mx = nc.dram_tensor("o_mx", [128, 16], F32, kind="ExternalOutput").ap()
    o_tl = nc.dram_tensor("o_tl", [1, Q], F32, kind="ExternalOutput").ap()

    with tile.TileContext(nc) as tc:
        for _rep in range(reps):
            _emit(nc, tc, din, o_se, o_mx, o_tl, sched, nmask, collective, _rep)

    nc.compile()
    nc.m = get_hw_module(nc.m)
    return nc


def _emit(nc, tc, din, o_se, o_mx, o_tl, sched, nmask, collective, rep):
    with tc.tile_pool(name=f"persist{rep}", bufs=1) as pp, \
         tc.tile_pool(name=f"dram{rep}", bufs=1, space="DRAM") as dp:
        # ---- loads: projection inputs first, masks next, lm-head weights last
        wq_sb = pp.tile([128, 256], F8, name="wq_sb")
        nc.sync.dma_start(wq_sb[:], din["i_wq"][:])
        wk_sb = pp.tile([128, 256], F8, name="wk_sb")
        nc.sync.dma_start(wk_sb[:], din["i_wk"][:])
        wv_sb = pp.tile([128, 256], F8, name="wv_sb")
        nc.sync.dma_start(wv_sb[:], din["i_wv"][:])
        xt = []
        for f in range(2):
            t = pp.tile([128, 2 * (S + Q)], F8, name=f"xt{f}")
            nc.sync.dma_start(t[:], din[f"i_xt{f}"][:])
            xt.append(t)
        bmask = pp.tile([128, max(1, nmask) * 512], F8, name="bmask")
        nc.sync.dma_start(bmask[:], din["i_bmask"][:])
        dmask = pp.tile([128, 512], F8, name="dmask")
        nc.sync.dma_start(dmask[:], din["i_dmask"][:])
        wo = []
        for f in range(2):
            t = pp.tile([128, 2 * D], F8, name=f"wo{f}")
            nc.sync.dma_start(t[:], din[f"i_wo{f}"][:])
            wo.append(t)
        wt_sb = pp.tile([128, 4 * Q], BF16, name="wt_sb")
        nc.sync.dma_start(wt_sb[:], din["i_wt"][:])
        wlm = []
        for f in range(2):
            t = pp.tile([128, 2 * VS], F8, name=f"wlm{f}")
            nc.sync.dma_start(t[:], din[f"i_wlm{f}"][:])
            wlm.append(t)

        xtv = [t.rearrange("p (j x) -> p j x", j=2) for t in xt]
        wqv = wq_sb.rearrange("p (f j d) -> p f j d", f=2, j=2)
        wkv = wk_sb.rearrange("p (f j d) -> p f j d", f=2, j=2)
        wvv = wv_sb.rearrange("p (f j d) -> p f j d", f=2, j=2)
        wov = [t.rearrange("p (j o) -> p j o", j=2) for t in wo]
        wlmv = [t.rearrange("p (j v) -> p j v", j=2) for t in wlm]
        wtv = wt_sb.rearrange("p (f q) -> p f q", f=4)

        ones64 = pp.tile([1, DH], BF16, name="ones64")
        nc.vector.memset(ones64[:], 1.0)
        onescol = pp.tile([128, 1], BF16, name="onescol")
        nc.vector.memset(onescol[:], 1.0)

        kT = pp.tile([DH, S + Q], BF16, name="kT")
        qT = pp.tile([DH, Q], BF16, name="qT")
        vaug = pp.tile([128, 32 * 68], F8, name="vaug")
        vav = vaug.rearrange("p (t c) -> p t c", c=68)
        nc.vector.memset(vav[:, :, DH:DH + 1], 2.0)   # denom ones-col (=2)
        nc.vector.memset(vav[:, :, DH + 1:68], 0.0)   # fp8-align padding
        gin = pp.tile([DH, Q], F8, name="gin")
        recip = pp.tile([1, Q], BF16, name="recip")
        bcs = pp.tile([DH, 512], BF16, name="bcs")
        ctxf = [pp.tile([128, 2 * Q], F8, name=f"ctxf{f}") for f in range(2)]
        ctxfv = [t.rearrange("p (j q) -> p j q", j=2) for t in ctxf]
        outT = [pp.tile([128, 2 * Q], F8, name=f"outT{f}") for f in range(2)]
        outTv = [t.rearrange("p (j q) -> p j q", j=2) for t in outT]
        se_sb = pp.tile([128, 16], F32, name="se_sb")
        mx_sb = pp.tile([128, 16], F32, name="mx_sb")
        tl_sb = pp.tile([1, Q], F32, name="tl_sb")
        gb_in = [dp.tile([DH, Q // 2], F8, name=f"gb_in{h}") for h in range(2)]
        gb_out = [dp.tile([NC * DH, Q // 2], F8, name=f"gb_out{h}",
                          addr_space="Shared" if collective else "Local")
                  for h in range(2)]

        # ---- projections (DoubleRow fp8)
        with tc.tile_pool(name=f"projps{rep}", bufs=2, space="PSUM") as projps:
            for n in (4, 0, 1, 5, 2, 6, 3, 7):
                ps = projps.tile([DH, 512], F32, name="kps", tag="proj")
                for f in range(2):
                    nc.tensor.matmul(ps[:], wkv[:, f], xtv[f][:, :, 512 * n:512 * (n + 1)],
                                     start=(f == 0), stop=(f == 1), perf_mode=DR)
                nc.scalar.mul(kT[:, 512 * n:512 * (n + 1)], ps[:], SK / (SX * SW))
            for n in range(Q // 512):
                ps = projps.tile([DH, 512], F32, name="qps", tag="proj")
                for f in range(2):
                    nc.tensor.matmul(ps[:], wqv[:, f],
                                     xtv[f][:, :, S + 512 * n:S + 512 * (n + 1)],
                                     start=(f == 0), stop=(f == 1), perf_mode=DR)
                nc.scalar.mul(qT[:, 512 * n:512 * (n + 1)], ps[:], SK / (SX * SW))
            for T in list(range(16, 32)) + list(range(16)):
                ps = projps.tile([128, DH], F32, name="vps", tag="proj")
                for f in range(2):
                    nc.tensor.matmul(ps[:], xtv[f][:, :, 128 * T:128 * (T + 1)],
                                     wvv[:, f], start=(f == 0), stop=(f == 1),
                                     perf_mode=DR)
                nc.vector.tensor_scalar(vav[:, T, 0:DH], ps[:], SV / (SX * SW),
                                        None, mybir.AluOpType.mult)

        # ---- attention: two-half pipeline with per-half AllGather
        with tc.tile_pool(name=f"scoreps{rep}", bufs=2, space="PSUM") as scoreps, \
             tc.tile_pool(name=f"ctxps{rep}", bufs=3, space="PSUM") as ctxps, \
             tc.tile_pool(name=f"bcps{rep}", bufs=1, space="PSUM") as bcps, \
             tc.tile_pool(name=f"abuf{rep}", bufs=4) as abuf:
            for half in range(2):
                for g in (2 * half, 2 * half + 1):
                    pairs = sched[g]
                    qs = 512 * g
                    cps = ctxps.tile([68, 512], F32, name="cps")
                    # draft tiles first (start=True per 128-col region)
                    dps = scoreps.tile([128, 512], F32, name="dsps", tag="sc")
                    for u in range(4):
                        t = 16 + 4 * g + u
                        nc.tensor.matmul(dps[:, 128 * u:128 * (u + 1)],
                                         kT[:, 128 * t:128 * (t + 1)],
                                         qT[:, qs + 128 * u:qs + 128 * (u + 1)],
                                         start=True, stop=True)
                    dp_sb = abuf.tile([128, 1024], F8, name="p_sb", tag="p")
                    nc.scalar.activation(dp_sb[:, 0:512], dps[:],
                                         mybir.ActivationFunctionType.Exp,
                                         scale=0.125 / (SK * SK))
                    nc.gpsimd.tensor_tensor(dp_sb[:, 0:512], dp_sb[:, 0:512],
                                            dmask[:], mybir.AluOpType.mult)
                    for u in range(4):
                        t = 16 + 4 * g + u
                        nc.tensor.matmul(cps[:, 128 * u:128 * (u + 1)],
                                         vav[:, t], dp_sb[:, 128 * u:128 * (u + 1)],
                                         start=True, stop=True,
                                         skip_group_check=True)
                    # ctx tiles in pairs, trimmed to live cols
                    for pi, pair in enumerate(pairs):
                        a0 = pair[0][2]
                        sps = scoreps.tile([128, 1024], F32, name="sps", tag="sc")
                        for m, (t, mtype, a, mi) in enumerate(pair):
                            nc.tensor.matmul(sps[:, 512 * m + a:512 * m + 512],
                                             kT[:, 128 * t:128 * (t + 1)],
                                             qT[:, qs + a:qs + 512],
                                             start=True, stop=True)
                        p_sb = abuf.tile([128, 1024], F8, name="p_sb", tag="p")
                        spv = sps.rearrange("p (m w) -> p m w", m=2)
                        ppv = p_sb.rearrange("p (m w) -> p m w", m=2)
                        nc.scalar.activation(ppv[:, 0:len(pair), a0:512],
                                             spv[:, 0:len(pair), a0:512],
                                             mybir.ActivationFunctionType.Exp,
                                             scale=0.125 / (SK * SK))
                        for m, (t, mtype, a, mi) in enumerate(pair):
                            if mtype:
                                pv = p_sb[:, 512 * m + a0:512 * m + 512]
                                nc.gpsimd.tensor_tensor(
                                    pv, pv, bmask[:, 512 * mi + a0:512 * mi + 512],
                                    mybir.AluOpType.mult)
                        for m, (t, mtype, a, mi) in enumerate(pair):
                            nc.tensor.matmul(cps[:, a0:512],
                                             vav[:, t],
                                             p_sb[:, 512 * m + a0:512 * m + 512],
                                             start=False,
                                             stop=(pi == len(pairs) - 1
                                                   and m == len(pair) - 1),
                                             skip_group_check=True)
                    # normalize: recip of (2*sum p) => ctx*SG via *32 values
                    with nc.allow_low_precision(reason="bf16 recip of denom"):
                        nc.vector.reciprocal(recip[:, qs:qs + 512],
                                             cps[DH:DH + 1, :])
                    bps = bcps.tile([DH, 512], F32, name="bps")
                    nc.tensor.matmul(bps[:], ones64[:], recip[:, qs:qs + 512],
                                     start=True, stop=True)
                    nc.vector.tensor_copy(bcs[:], bps[:])
                    nc.vector.tensor_tensor(gin[:, qs:qs + 512], cps[0:DH, :],
                                            bcs[:], mybir.AluOpType.mult)
                # AllGather for this half
                hs_ = slice(1024 * half, 1024 * (half + 1))
                nc.sync.dma_start(gb_in[half][:], gin[:, hs_])
                if collective:
                    nc.gpsimd.collective_compute(
                        "AllGather", mybir.AluOpType.bypass,
                        replica_groups=[list(range(NC))],
                        ins=[gb_in[half].opt()], outs=[gb_out[half].opt()])
                else:  # timing-model variant: fake the gather with local DMAs
                    for _c in range(NC):
                        nc.sync.dma_start(gb_out[half][DH * _c:DH * (_c + 1), :],
                                          gb_in[half][:])
                for f in range(2):
                    for j in range(2):
                        nc.sync.dma_start(
                            ctxfv[f][:, j, 1024 * half:1024 * (half + 1)],
                            gb_out[half][256 * f + 128 * j:256 * f + 128 * (j + 1), :])

        # ---- Wo (DoubleRow fp8) + target logits, all 4 groups
        with tc.tile_pool(name=f"wops{rep}", bufs=2, space="PSUM") as wops, \
             tc.tile_pool(name=f"tlps{rep}", bufs=2, space="PSUM") as tlps, \
             tc.tile_pool(name=f"stbuf{rep}", bufs=2) as stbuf:
            for g in range(QG):
                for fo in range(4):
                    ps = wops.tile([128, 512], F32, name="wps")
                    for f in range(2):
                        nc.tensor.matmul(
                            ps[:], wov[f][:, :, 128 * fo:128 * (fo + 1)],
                            ctxfv[f][:, :, 512 * g:512 * (g + 1)],
                            start=(f == 0), stop=(f == 1), perf_mode=DR)
                    nc.vector.tensor_scalar(
                        outTv[fo // 2][:, fo % 2, 512 * g:512 * (g + 1)],
                        ps[:], SO / (SG * SWO), None, mybir.AluOpType.mult)
            for g in range(QG):
                ps = tlps.tile([1, 512], F32, name="tlp")
                for fo in range(4):
                    mmc = stbuf.tile([128, 512], BF16, name="mmc", tag="mmc")
                    nc.gpsimd.tensor_tensor(
                        mmc[:], outTv[fo // 2][:, fo % 2, 512 * g:512 * (g + 1)],
                        wtv[:, fo, 512 * g:512 * (g + 1)], mybir.AluOpType.mult)
                    nc.tensor.matmul(ps[:], onescol[:], mmc[:],
                                     start=(fo == 0), stop=(fo == 3))
                nc.vector.tensor_copy(tl_sb[:, 512 * g:512 * (g + 1)], ps[:])

        # ---- LM head: 16 q-chunks x 4000 vocab, fp8 DoubleRow, exp+accum,
        #      bf16 tree-max (DVE 2x; Pool helps on odd chunks)
        with tc.tile_pool(name=f"zps{rep}", bufs=2, space="PSUM") as zps, \
             tc.tile_pool(name=f"zbuf{rep}", bufs=4) as zbuf, \
             tc.tile_pool(name=f"mxbuf{rep}", bufs=4) as mxbuf, \
             tc.tile_pool(name=f"stbuf2{rep}", bufs=2) as stbuf2:
            for i in range(16):
                se2 = stbuf2.tile([128, 2], F32, name="se2", tag="se2")
                zes = []
                for c2 in range(2):
                    ps = zps.tile([128, 2048], F32, name="zp")
                    for f in range(2):
                        for h in range(4):
                            nc.tensor.matmul(
                                ps[:, 512 * h:512 * h + ZC],
                                outTv[f][:, :, 128 * i:128 * (i + 1)],
                                wlmv[f][:, :, 2000 * c2 + ZC * h:2000 * c2 + ZC * (h + 1)],
                                start=(f == 0), stop=(f == 1),
                                perf_mode=DR, skip_group_check=True)
                    ze = zbuf.tile([128, 2048], BF16, name="ze")
                    psv = ps.rearrange("p (c w) -> p c w", w=512)[:, :, 0:ZC]
                    zev = ze.rearrange("p (c w) -> p c w", w=512)[:, :, 0:ZC]
                    nc.scalar.activation(zev, psv,
                                         mybir.ActivationFunctionType.Exp,
                                         scale=1.0 / (SO * SL),
                                         accum_out=se2[:, c2:c2 + 1])
                    zes.append(ze)
                nc.vector.tensor_tensor(se_sb[:, i:i + 1], se2[:, 0:1], se2[:, 1:2],
                                        mybir.AluOpType.add)
                # tree max over 2x[128, 4x500(str512)] bf16
                zm = [mxbuf.tile([128, 1024], BF16, name="zm", tag="zm")
                      for _ in range(2)]
                for c2 in range(2):
                    zv = zes[c2].rearrange("p (c w) -> p c w", w=512)[:, :, 0:ZC]
                    zmv = zm[c2].rearrange("p (c w) -> p c w", w=512)[:, :, 0:ZC]
                    nc.vector.tensor_tensor(zmv, zv[:, 0:2], zv[:, 2:4],
                                            mybir.AluOpType.max)
                z3 = zm[0].rearrange("p (c w) -> p c w", w=512)[:, :, 0:ZC]
                z4 = zm[1].rearrange("p (c w) -> p c w", w=512)[:, :, 0:ZC]
                nc.vector.tensor_tensor(z3, z3, z4, mybir.AluOpType.max)
                nc.vector.tensor_tensor(zm[0][:, 0:ZC], zm[0][:, 0:ZC],
                                        zm[0][:, 512:512 + ZC],
                                        mybir.AluOpType.max)
                nc.vector.tensor_reduce(mx_sb[:, i:i + 1], zm[0][:, 0:ZC],
                                        mybir.AxisListType.X, mybir.AluOpType.max)
        nc.sync.dma_start(o_tl[:], tl_sb[:])
        nc.sync.dma_start(o_se[:], se_sb[:])
        nc.sync.dma_start(o_mx[:], mx_sb[:])


def _pack2(a, scale):
    """[512, X] f32 -> 2 fp8 arrays [128, 2*X]: arr_fp[p, j*X+x] = a[256fp+128j+p, x]."""
    x = a.shape[1]
    r = (a * scale).reshape(2, 2, 128, x).astype(F8NP)
    return [np.ascontiguousarray(r[fp].transpose(1, 0, 2).reshape(128, 2 * x))
            for fp in range(2)]


def kernel(**inputs):
    ids = np.asarray(inputs["input_ids"])[0].astype(np.int64)        # [S]
    hs = np.asarray(inputs["hidden_states"])[0].astype(np.float32)   # [S, D]
    lmask = np.asarray(inputs["loss_mask"])[0].astype(np.float32)    # [S]
    anc = np.asarray(inputs["anchor_positions"])[0].astype(np.int64)  # [N]
    keep = np.asarray(inputs["block_keep_mask"])[0].astype(bool)     # [N]
    emb = np.asarray(inputs["embed_table"]).astype(np.float32)       # [V, D]
    Wq = np.asarray(inputs["Wq"]).astype(np.float32)
    Wk = np.asarray(inputs["Wk"]).astype(np.float32)
    Wv = np.asarray(inputs["Wv"]).astype(np.float32)
    Wo = np.asarray(inputs["Wo"]).astype(np.float32)
    Wlm = np.asarray(inputs["W_lm"]).astype(np.float32)

    # ---- host layout prep ----
    safe_anchor = np.clip(anc, 0, S - 1)
    start_tokens = np.where(keep, ids[safe_anchor], MASK_TOKEN_ID)
    ne = np.tile(emb[MASK_TOKEN_ID], (Q, 1))
    ne[0::BS] = emb[start_tokens]                   # [Q, D]
    xt_full = np.concatenate([hs, ne], 0).T         # [D, S+Q]

    offs = np.arange(BS)
    label_idx = anc[:, None] + offs[None, :]
    valid = (label_idx < S)
    safe_idx = np.clip(label_idx, 0, S - 1)
    targets = ids[safe_idx].reshape(-1)             # [Q]
    w = (keep[:, None] * valid * (offs > 0)[None, :]
         * lmask[safe_idx]).astype(np.float32).reshape(-1)

    xt8 = _pack2(xt_full, SX)
    p_idx = np.arange(128)[:, None]
    j_idx = np.arange(512)[None, :]
    dmask = ((p_idx // BS) == (j_idx % 128) // BS).astype(np.float32).astype(F8NP)
    sched, nmask = _build_schedule(anc)
    anchor_q = np.repeat(anc, BS)                    # [Q]
    bmask = np.zeros((128, max(1, nmask) * 512), np.float32)
    for g in range(QG):
        for pair in sched[g]:
            for (t, mtype, a, mi) in pair:
                if mtype:
                    kv = 128 * t + np.arange(128)[:, None]
                    av = anchor_q[None, 512 * g:512 * (g + 1)]
                    bmask[:, 512 * mi:512 * (mi + 1)] = (av > kv)
    bmask = bmask.astype(F8NP)
    wt = np.ascontiguousarray(
        (Wlm[:, targets] / SO).reshape(4, 128, Q).transpose(1, 0, 2)
        .reshape(128, 4 * Q)).astype(BFNP)
    wo8 = _pack2(Wo, SWO)
    wlm8 = {}
    for c in range(NC):
        wlm8[c] = _pack2(Wlm[:, VS * c:VS * (c + 1)], SL)

    key = (anc.tobytes(), 3)
    if key not in _cache:
        _cache[key] = _build_program(sched, nmask)
    nc = _cache[key]

    in_maps = []
    for c in range(NC):
        wq8 = _pack2(Wq[:, DH * c:DH * (c + 1)], SW)
        wk8 = _pack2(Wk[:, DH * c:DH * (c + 1)], SW)
        wv8 = _pack2(Wv[:, DH * c:DH * (c + 1)], SW)
        in_maps.append({
            "i_bmask": bmask, "i_dmask": dmask,
            "i_wq": np.concatenate(wq8, 1), "i_wk": np.concatenate(wk8, 1),
            "i_wv": np.concatenate(wv8, 1),
            "i_xt0": xt8[0], "i_xt1": xt8[1],
            "i_wo0": wo8[0], "i_wo1": wo8[1],
            "i_wt": wt,
            "i_wlm0": wlm8[c][0], "i_wlm1": wlm8[c][1],
        })

    global _last_in_maps
    _last_in_maps = in_maps
    res = run_bass_kernel_spmd(nc, in_maps, core_ids=list(range(NC)))

    # ---- host combine ----
    se = np.zeros((128, 16), np.float64)
    mx = np.zeros((128, 16), np.float32)
    for c in range(NC):
        se += res.results[c]["o_se"].astype(np.float64)
        mx = np.maximum(mx, res.results[c]["o_mx"])
    se_q = se.T.reshape(-1)           # q = 128*i + p
    mx_q = mx.T.reshape(-1)           # max of exp(z)
    tl_q = res.results[0]["o_tl"][0]

    lse = np.log(se_q)
    loss_per = np.where(w > 0, lse - tl_q, 0.0)
    loss = (loss_per * w).sum() / (w.sum() + 1e-6)
    correct = (tl_q >= np.log(np.maximum(mx_q, 1e-30)) - 3e-4) & (w > 0.5)
    acc = correct.sum() / (w.sum() + 1e-6)
    return np.float32(loss), np.float32(acc)
